# revision 18
# baseline (speedup 1.0000x reference)
"""Band VQ forward on 8 Trainium2 NeuronCores.

Problem: x [B=8, NB=3, D=512, T=2048] f32, codebook [NB=3, K=1024, D=512] f32.
Returns (quantized [B,NB,D,T] f32, codes [B,NB,T] int32, commit_loss scalar).

Sharding: data-parallel over batch B -> one batch per core; codebooks
replicated on every core; commit loss reduced from per-core partials on host.

Per-core device pipeline (per band, per 128-frame tile):
  - scores[t,k] = (2x).e  via PE matmuls. fp32 accuracy at tf32 speed using a
    3-term hi/lo split (a_hi*b_hi + a_hi*b_lo + a_lo*b_hi) in float32r mode
    (1 cyc/row vs 4 for fp32; measured ~1e-6 rel err). The hi/lo split is done
    ON DEVICE (f32->f32r cast rounds to the 11-bit tf32 mantissa; lo = x - hi)
    so x and the codebook ship over DMA once in plain fp32. The 2x scale is
    folded into x host-side.
  - scores -= |e|^2 broadcast (DVE), argmax via DVE max/max_index
    (argmin of distance = argmax of score)
  - gather codeword rows with indirect DMA, PE-transpose to channels-first.
    Transposes/copies for tile i are emitted after the matmuls of tile i+DEFER
    so the PE never stalls on the argmax->gather chain.
  - commit loss via identity sum((q-x)^2) = sum(x^2) - sum(max_score):
    sum(x^2) in f64 on host during input prep; per-frame max scores shipped
    back.
"""
import numpy as np
from contextlib import ExitStack

import concourse.bass as bass
import concourse.tile as tile
from concourse import bacc, mybir
from concourse import bass_utils
from concourse.masks import make_identity

B, NB, D, T, K = 8, 3, 512, 2048, 1024
P = 128
NDC = D // P        # 4 d-chunks of 128
TCH = 1024          # q assembly/output chunk (4KB contiguous output runs)
XCH = 512           # x load/split chunk
KC = 512            # k half (one PSUM bank)
DEFER = 8           # tiles of slack between matmuls and PE transposes

f32 = mybir.dt.float32
f32r = mybir.dt.float32r
u32 = mybir.dt.uint32
i32 = mybir.dt.int32

TRACE = False        # test.py can flip this to capture an NTFF profile
LAST = {}            # test.py introspection (exec_time_ns etc.)


def build_nc(t_total: int = T):
    """Build the per-core Bass program. t_total lets tests build a smaller
    variant for CoreSim."""
    tch = min(TCH, t_total)
    ntt = tch // P            # 128-frame tiles per q chunk
    xch = min(XCH, t_total)   # x load chunk
    nxc = t_total // xch

    nc = bacc.Bacc("TRN2", target_bir_lowering=False, debug=False)

    x_d = nc.dram_tensor("x2", [NB, D, t_total], f32, kind="ExternalInput").ap()
    b_d = nc.dram_tensor("bt", [NB, D, K], f32, kind="ExternalInput").ap()
    e2_d = nc.dram_tensor("e2", [NB, K], f32, kind="ExternalInput").ap()
    cb_d = [
        nc.dram_tensor(f"cb{n}", [K, D], f32, kind="ExternalInput").ap()
        for n in range(NB)
    ]

    q_d = nc.dram_tensor("q", [NB, D, t_total], f32, kind="ExternalOutput").ap()
    codes_d = nc.dram_tensor("codes", [NB, t_total], i32, kind="ExternalOutput").ap()
    smax_d = nc.dram_tensor("smax", [NB, t_total], f32, kind="ExternalOutput").ap()

    x_r = x_d.rearrange("n (c p) t -> p n c t", p=P)
    q_r = q_d.rearrange("n (c p) t -> p n c t", p=P)

    with tile.TileContext(nc) as tc, ExitStack() as ctx:
        const = ctx.enter_context(tc.tile_pool(name="const", bufs=1))
        bspool = ctx.enter_context(tc.tile_pool(name="bspool", bufs=1))
        bpool = ctx.enter_context(tc.tile_pool(name="bpool", bufs=2))
        xspool = ctx.enter_context(tc.tile_pool(name="xspool", bufs=2))
        xpool = ctx.enter_context(tc.tile_pool(name="xpool", bufs=2))
        qpool = ctx.enter_context(tc.tile_pool(name="qpool", bufs=2))
        scpool = ctx.enter_context(tc.tile_pool(name="scpool", bufs=3))
        qrpool = ctx.enter_context(tc.tile_pool(name="qrpool", bufs=10))
        mpool = ctx.enter_context(tc.tile_pool(name="mpool", bufs=8))
        pspool = ctx.enter_context(tc.tile_pool(name="pspool", bufs=3, space="PSUM"))
        pqpool = ctx.enter_context(tc.tile_pool(name="pqpool", bufs=2, space="PSUM"))

        ident = const.tile([P, P], f32)
        make_identity(nc, ident[:])

        e2pool = ctx.enter_context(tc.tile_pool(name="e2pool", bufs=2))

        q_tiles = {}      # (band, tci) -> [tile, writes_done]
        pending = []      # deferred (band, tci, tt, qr_tile)
        final_key = (NB - 1, t_total // tch - 1)

        def flush_one():
            band, tci, tt, qr = pending.pop(0)
            key = (band, tci)
            if key not in q_tiles:
                q_tiles[key] = [
                    qpool.tile([P, NDC, tch], f32, tag="qtc",
                               name=f"qtc_{band}_{tci}"), 0]
            q_tc, done = q_tiles[key]
            pq = pqpool.tile([P, D], f32, tag="pq", name=f"pq_{band}_{tci}_{tt}")
            for c in range(NDC):
                nc.tensor.transpose(
                    pq[:, c * P : (c + 1) * P],
                    qr[:, c * P : (c + 1) * P],
                    ident[:],
                )
            tt0 = tt * P
            nc.scalar.copy(
                out=q_tc[:, :, tt0 : tt0 + P],
                in_=pq[:].rearrange("p (c t) -> p c t", c=NDC),
            )
            q_tiles[key][1] = done + 1
            t0 = tci * tch
            if key == final_key:
                # final chunk: stream the output in halves so the last DMA
                # overlaps the tail transposes instead of serializing after
                half = tch // 2
                if q_tiles[key][1] == ntt // 2:
                    nc.sync.dma_start(
                        q_r[:, band, :, t0 : t0 + half], q_tc[:, :, 0:half]
                    )
                elif q_tiles[key][1] == ntt:
                    nc.sync.dma_start(
                        q_r[:, band, :, t0 + half : t0 + tch],
                        q_tc[:, :, half:tch],
                    )
                    del q_tiles[key]
            elif q_tiles[key][1] == ntt:
                nc.sync.dma_start(q_r[:, band, :, t0 : t0 + tch], q_tc[:])
                del q_tiles[key]

        for band in range(NB):
            # load codebook fp32 in k-halves, split hi/lo on device per half
            # so the first matmuls only wait on the k0 half
            e2bc = e2pool.tile([P, K], f32, tag="e2bc", name=f"e2bc_{band}")
            nc.sync.dma_start(e2bc[:], e2_d[band][None].to_broadcast([P, K]))

            bfp = bspool.tile([P, NDC, K], f32, tag="bfp")
            bh_t = bpool.tile([P, NDC, K], f32r, tag="bh")
            bl_t = bpool.tile([P, NDC, K], f32r, tag="bl")
            b_rr = b_d[band].rearrange("(c p) k -> p c k", p=P)
            for kh in range(2):
                ks = slice(kh * KC, (kh + 1) * KC)
                nc.sync.dma_start(bfp[:, :, ks], b_rr[:, :, ks])
                nc.vector.tensor_copy(bh_t[:, :, ks], bfp[:, :, ks])
                nc.vector.tensor_tensor(
                    out=bl_t[:, :, ks], in0=bfp[:, :, ks],
                    in1=bh_t[:, :, ks].bitcast(f32),
                    op=mybir.AluOpType.subtract,
                )

            for xci in range(nxc):
                x0 = xci * xch
                xfp = xspool.tile([P, NDC, xch], f32, tag="xfp")
                nc.sync.dma_start(xfp[:], x_r[:, band, :, x0 : x0 + xch])
                xh_t = xpool.tile([P, NDC, xch], f32r, tag="xh")
                nc.scalar.copy(xh_t[:], xfp[:])
                xl_t = xpool.tile([P, NDC, xch], f32r, tag="xl")
                nc.vector.tensor_tensor(
                    out=xl_t[:], in0=xfp[:], in1=xh_t[:].bitcast(f32),
                    op=mybir.AluOpType.subtract,
                )

                for tti in range(xch // P):          # 128-frame tiles in x chunk
                    tg = x0 + tti * P                # global t offset
                    tci = tg // tch
                    tt = (tg % tch) // P
                    tt0 = tti * P

                    ps = pspool.tile([P, K], f32, tag="ps")
                    for dc in range(NDC):
                        lh = xh_t[:, dc, tt0 : tt0 + P]
                        ll = xl_t[:, dc, tt0 : tt0 + P]
                        st = dc == 0
                        sp = dc == NDC - 1
                        nc.tensor.matmul(ps[:, 0:KC], lh, bh_t[:, dc, 0:KC],
                                         start=st, stop=False)
                        nc.tensor.matmul(ps[:, KC:K], lh, bh_t[:, dc, KC:K],
                                         start=st, stop=False)
                        nc.tensor.matmul(ps[:, 0:KC], lh, bl_t[:, dc, 0:KC],
                                         start=False, stop=False)
                        nc.tensor.matmul(ps[:, KC:K], lh, bl_t[:, dc, KC:K],
                                         start=False, stop=False)
                        nc.tensor.matmul(ps[:, 0:KC], ll, bh_t[:, dc, 0:KC],
                                         start=False, stop=sp)
                        nc.tensor.matmul(ps[:, KC:K], ll, bh_t[:, dc, KC:K],
                                         start=False, stop=sp)

                    sc = scpool.tile([P, K], f32, tag="sc")
                    nc.vector.tensor_tensor(
                        out=sc[:], in0=ps[:], in1=e2bc[:],
                        op=mybir.AluOpType.subtract,
                    )
                    mx = mpool.tile([P, 8], f32, tag="mx")
                    mi = mpool.tile([P, 8], u32, tag="mi")
                    nc.vector.max(mx[:], sc[:])
                    nc.vector.max_index(mi[:], mx[:], sc[:])

                    nc.sync.dma_start(
                        codes_d[band, tg : tg + P, None], mi[:, 0:1].bitcast(i32)
                    )
                    nc.sync.dma_start(smax_d[band, tg : tg + P, None], mx[:, 0:1])

                    qr = qrpool.tile([P, D], f32, tag="qr")
                    nc.gpsimd.indirect_dma_start(
                        out=qr[:],
                        out_offset=None,
                        in_=cb_d[band],
                        in_offset=bass.IndirectOffsetOnAxis(ap=mi[:, 0:1], axis=0),
                    )

                    pending.append((band, tci, tt, qr))
                    if len(pending) > DEFER:
                        flush_one()

        while pending:
            flush_one()

    nc.compile()
    return nc


_NC_CACHE = {}


def _get_nc(t_total: int = T):
    if t_total not in _NC_CACHE:
        _NC_CACHE[t_total] = build_nc(t_total)
    return _NC_CACHE[t_total]


def prep_inputs(x: np.ndarray, codebook: np.ndarray):
    """Host-side prep: 2x scale, codebook transpose, |e|^2, per-core maps,
    sum(x^2)."""
    x = np.ascontiguousarray(x, dtype=np.float32)
    codebook = np.ascontiguousarray(codebook, dtype=np.float32)

    cbt = np.ascontiguousarray(codebook.transpose(0, 2, 1))  # [NB, D, K]
    e2 = (codebook.astype(np.float64) ** 2).sum(-1).astype(np.float32)  # [NB, K]

    shared = {"bt": cbt, "e2": e2}
    for n in range(NB):
        shared[f"cb{n}"] = np.ascontiguousarray(codebook[n])

    in_maps = []
    for b in range(B):
        in_maps.append({"x2": 2.0 * x[b], **shared})

    sum_x2 = float((x.astype(np.float64) ** 2).sum())
    return in_maps, sum_x2


def kernel(x: np.ndarray, codebook: np.ndarray):
    x = np.asarray(x)
    codebook = np.asarray(codebook)
    in_maps, sum_x2 = prep_inputs(x, codebook)
    nc = _get_nc()

    res = bass_utils.run_bass_kernel_spmd(
        nc, in_maps, core_ids=list(range(B)), trace=TRACE
    )
    LAST["exec_time_ns"] = res.exec_time_ns
    LAST["profile_json"] = res.profile_json

    outs = res.results
    quantized = np.stack([outs[b]["q"] for b in range(B)])        # [B,NB,D,T]
    codes = np.stack([outs[b]["codes"] for b in range(B)])        # [B,NB,T]
    smax = np.stack([outs[b]["smax"] for b in range(B)])          # [B,NB,T]

    sum_smax = float(smax.astype(np.float64).sum())
    commit_loss = np.float32((sum_x2 - sum_smax) / (B * NB * T * D))
    return quantized, codes, commit_loss


# revision 19
# speedup vs baseline: 1.1984x; 1.1984x over previous
"""Band VQ forward on 8 Trainium2 NeuronCores.

Problem: x [B=8, NB=3, D=512, T=2048] f32, codebook [NB=3, K=1024, D=512] f32.
Returns (quantized [B,NB,D,T] f32, codes [B,NB,T] int32, commit_loss scalar).

Sharding: data-parallel over batch B -> one batch per core; codebooks
replicated on every core; commit loss reduced from per-core partials on host.

Per-core device pipeline (per band, per 128-frame tile):
  - scores[t,k] = (2x).e  via PE matmuls. fp32 accuracy at tf32 speed using a
    3-term hi/lo split (a_hi*b_hi + a_hi*b_lo + a_lo*b_hi) in float32r mode
    (1 cyc/row vs 4 for fp32; measured ~1e-6 rel err). The hi/lo split is done
    ON DEVICE (f32->f32r cast rounds to the 11-bit tf32 mantissa; lo = x - hi)
    so x and the codebook ship over DMA once in plain fp32. The 2x scale is
    folded into x host-side.
  - scores -= |e|^2 broadcast (DVE), argmax via DVE max/max_index
    (argmin of distance = argmax of score)
  - gather codeword rows with indirect DMA, PE-transpose to channels-first.
    Transposes/copies for tile i are emitted after the matmuls of tile i+DEFER
    so the PE never stalls on the argmax->gather chain.
  - commit loss via identity sum((q-x)^2) = sum(x^2) - sum(max_score):
    sum(x^2) in f64 on host during input prep; per-frame max scores shipped
    back.
"""
import numpy as np
from contextlib import ExitStack

import concourse.bass as bass
import concourse.tile as tile
from concourse import bacc, mybir
from concourse import bass_utils
from concourse.masks import make_identity

B, NB, D, T, K = 8, 3, 512, 2048, 1024
P = 128
NDC = D // P        # 4 d-chunks of 128
TCH = 1024          # q assembly/output chunk (4KB contiguous output runs)
XCH = 512           # x load/split chunk
KC = 512            # k half (one PSUM bank)
DEFER = 8           # tiles of slack between matmuls and PE transposes

f32 = mybir.dt.float32
f32r = mybir.dt.float32r
u32 = mybir.dt.uint32
i32 = mybir.dt.int32

TRACE = False        # test.py can flip this to capture an NTFF profile
LAST = {}            # test.py introspection (exec_time_ns etc.)


def build_nc(t_total: int = T):
    """Build the per-core Bass program. t_total lets tests build a smaller
    variant for CoreSim."""
    tch = min(TCH, t_total)
    ntt = tch // P            # 128-frame tiles per q chunk
    xch = min(XCH, t_total)   # x load chunk
    nxc = t_total // xch

    nc = bacc.Bacc("TRN2", target_bir_lowering=False, debug=False)

    x_d = nc.dram_tensor("x2", [NB, D, t_total], f32, kind="ExternalInput").ap()
    b_d = nc.dram_tensor("bt", [NB, D, K], f32, kind="ExternalInput").ap()
    e2_d = nc.dram_tensor("e2", [NB, K], f32, kind="ExternalInput").ap()
    cb_d = [
        nc.dram_tensor(f"cb{n}", [K, D], f32, kind="ExternalInput").ap()
        for n in range(NB)
    ]

    q_d = nc.dram_tensor("q", [NB, D, t_total], f32, kind="ExternalOutput").ap()
    codes_d = nc.dram_tensor("codes", [NB, t_total], i32, kind="ExternalOutput").ap()
    smax_d = nc.dram_tensor("smax", [NB, t_total], f32, kind="ExternalOutput").ap()

    x_r = x_d.rearrange("n (c p) t -> p n c t", p=P)
    q_r = q_d.rearrange("n (c p) t -> p n c t", p=P)

    with tile.TileContext(nc) as tc, ExitStack() as ctx:
        const = ctx.enter_context(tc.tile_pool(name="const", bufs=1))
        bspool = ctx.enter_context(tc.tile_pool(name="bspool", bufs=1))
        bpool = ctx.enter_context(tc.tile_pool(name="bpool", bufs=2))
        xspool = ctx.enter_context(tc.tile_pool(name="xspool", bufs=2))
        xpool = ctx.enter_context(tc.tile_pool(name="xpool", bufs=2))
        qpool = ctx.enter_context(tc.tile_pool(name="qpool", bufs=2))
        scpool = ctx.enter_context(tc.tile_pool(name="scpool", bufs=3))
        qrpool = ctx.enter_context(tc.tile_pool(name="qrpool", bufs=10))
        mpool = ctx.enter_context(tc.tile_pool(name="mpool", bufs=8))
        pspool = ctx.enter_context(tc.tile_pool(name="pspool", bufs=3, space="PSUM"))
        pqpool = ctx.enter_context(tc.tile_pool(name="pqpool", bufs=2, space="PSUM"))

        ident = const.tile([P, P], f32)
        make_identity(nc, ident[:])

        e2pool = ctx.enter_context(tc.tile_pool(name="e2pool", bufs=2))

        q_tiles = {}      # (band, tci) -> [tile, writes_done]
        pending = []      # deferred (band, tci, tt, qr_tile)

        def flush_one():
            band, tci, tt, qr = pending.pop(0)
            key = (band, tci)
            if key not in q_tiles:
                q_tiles[key] = [
                    qpool.tile([P, NDC, tch], f32, tag="qtc",
                               name=f"qtc_{band}_{tci}"), 0]
            q_tc, done = q_tiles[key]
            pq = pqpool.tile([P, D], f32, tag="pq", name=f"pq_{band}_{tci}_{tt}")
            for c in range(NDC):
                nc.tensor.transpose(
                    pq[:, c * P : (c + 1) * P],
                    qr[:, c * P : (c + 1) * P],
                    ident[:],
                )
            tt0 = tt * P
            nc.scalar.copy(
                out=q_tc[:, :, tt0 : tt0 + P],
                in_=pq[:].rearrange("p (c t) -> p c t", c=NDC),
            )
            q_tiles[key][1] = done + 1
            if q_tiles[key][1] == ntt:
                t0 = tci * tch
                nc.sync.dma_start(q_r[:, band, :, t0 : t0 + tch], q_tc[:])
                del q_tiles[key]

        for band in range(NB):
            # load codebook fp32 in k-halves, split hi/lo on device per half
            # so the first matmuls only wait on the k0 half
            e2bc = e2pool.tile([P, K], f32, tag="e2bc", name=f"e2bc_{band}")
            nc.sync.dma_start(e2bc[:], e2_d[band][None].to_broadcast([P, K]))

            bfp = bspool.tile([P, NDC, K], f32, tag="bfp")
            bh_t = bpool.tile([P, NDC, K], f32r, tag="bh")
            bl_t = bpool.tile([P, NDC, K], f32r, tag="bl")
            b_rr = b_d[band].rearrange("(c p) k -> p c k", p=P)
            for kh in range(2):
                ks = slice(kh * KC, (kh + 1) * KC)
                nc.sync.dma_start(bfp[:, :, ks], b_rr[:, :, ks])
                nc.vector.tensor_copy(bh_t[:, :, ks], bfp[:, :, ks])
                nc.vector.tensor_tensor(
                    out=bl_t[:, :, ks], in0=bfp[:, :, ks],
                    in1=bh_t[:, :, ks].bitcast(f32),
                    op=mybir.AluOpType.subtract,
                )

            for xci in range(nxc):
                x0 = xci * xch
                xfp = xspool.tile([P, NDC, xch], f32, tag="xfp")
                nc.sync.dma_start(xfp[:], x_r[:, band, :, x0 : x0 + xch])
                xh_t = xpool.tile([P, NDC, xch], f32r, tag="xh")
                nc.scalar.copy(xh_t[:], xfp[:])
                xl_t = xpool.tile([P, NDC, xch], f32r, tag="xl")
                nc.vector.tensor_tensor(
                    out=xl_t[:], in0=xfp[:], in1=xh_t[:].bitcast(f32),
                    op=mybir.AluOpType.subtract,
                )

                for tti in range(xch // P):          # 128-frame tiles in x chunk
                    tg = x0 + tti * P                # global t offset
                    tci = tg // tch
                    tt = (tg % tch) // P
                    tt0 = tti * P

                    ps = pspool.tile([P, K], f32, tag="ps")
                    for dc in range(NDC):
                        lh = xh_t[:, dc, tt0 : tt0 + P]
                        ll = xl_t[:, dc, tt0 : tt0 + P]
                        st = dc == 0
                        sp = dc == NDC - 1
                        nc.tensor.matmul(ps[:, 0:KC], lh, bh_t[:, dc, 0:KC],
                                         start=st, stop=False)
                        nc.tensor.matmul(ps[:, KC:K], lh, bh_t[:, dc, KC:K],
                                         start=st, stop=False)
                        nc.tensor.matmul(ps[:, 0:KC], lh, bl_t[:, dc, 0:KC],
                                         start=False, stop=False)
                        nc.tensor.matmul(ps[:, KC:K], lh, bl_t[:, dc, KC:K],
                                         start=False, stop=False)
                        nc.tensor.matmul(ps[:, 0:KC], ll, bh_t[:, dc, 0:KC],
                                         start=False, stop=sp)
                        nc.tensor.matmul(ps[:, KC:K], ll, bh_t[:, dc, KC:K],
                                         start=False, stop=sp)

                    sc = scpool.tile([P, K], f32, tag="sc")
                    nc.vector.tensor_tensor(
                        out=sc[:], in0=ps[:], in1=e2bc[:],
                        op=mybir.AluOpType.subtract,
                    )
                    mx = mpool.tile([P, 8], f32, tag="mx")
                    mi = mpool.tile([P, 8], u32, tag="mi")
                    nc.vector.max(mx[:], sc[:])
                    nc.vector.max_index(mi[:], mx[:], sc[:])

                    nc.sync.dma_start(
                        codes_d[band, tg : tg + P, None], mi[:, 0:1].bitcast(i32)
                    )
                    nc.sync.dma_start(smax_d[band, tg : tg + P, None], mx[:, 0:1])

                    qr = qrpool.tile([P, D], f32, tag="qr")
                    nc.gpsimd.indirect_dma_start(
                        out=qr[:],
                        out_offset=None,
                        in_=cb_d[band],
                        in_offset=bass.IndirectOffsetOnAxis(ap=mi[:, 0:1], axis=0),
                    )

                    pending.append((band, tci, tt, qr))
                    if len(pending) > DEFER:
                        flush_one()

        while pending:
            flush_one()

    nc.compile()
    return nc


_NC_CACHE = {}


def _get_nc(t_total: int = T):
    if t_total not in _NC_CACHE:
        _NC_CACHE[t_total] = build_nc(t_total)
    return _NC_CACHE[t_total]


def prep_inputs(x: np.ndarray, codebook: np.ndarray):
    """Host-side prep: 2x scale, codebook transpose, |e|^2, per-core maps,
    sum(x^2)."""
    x = np.ascontiguousarray(x, dtype=np.float32)
    codebook = np.ascontiguousarray(codebook, dtype=np.float32)

    cbt = np.ascontiguousarray(codebook.transpose(0, 2, 1))  # [NB, D, K]
    e2 = (codebook.astype(np.float64) ** 2).sum(-1).astype(np.float32)  # [NB, K]

    shared = {"bt": cbt, "e2": e2}
    for n in range(NB):
        shared[f"cb{n}"] = np.ascontiguousarray(codebook[n])

    in_maps = []
    for b in range(B):
        in_maps.append({"x2": 2.0 * x[b], **shared})

    sum_x2 = float((x.astype(np.float64) ** 2).sum())
    return in_maps, sum_x2


def kernel(x: np.ndarray, codebook: np.ndarray):
    x = np.asarray(x)
    codebook = np.asarray(codebook)
    in_maps, sum_x2 = prep_inputs(x, codebook)
    nc = _get_nc()

    res = bass_utils.run_bass_kernel_spmd(
        nc, in_maps, core_ids=list(range(B)), trace=TRACE
    )
    LAST["exec_time_ns"] = res.exec_time_ns
    LAST["profile_json"] = res.profile_json

    outs = res.results
    quantized = np.stack([outs[b]["q"] for b in range(B)])        # [B,NB,D,T]
    codes = np.stack([outs[b]["codes"] for b in range(B)])        # [B,NB,T]
    smax = np.stack([outs[b]["smax"] for b in range(B)])          # [B,NB,T]

    sum_smax = float(smax.astype(np.float64).sum())
    commit_loss = np.float32((sum_x2 - sum_smax) / (B * NB * T * D))
    return quantized, codes, commit_loss
